# revision 33
# baseline (speedup 1.0000x reference)
"""Multihead attention (B=2, L=2048, D=1024, 16 heads) on 8 trn2 cores.

Sharding: tensor-parallel over heads — 2 heads per core. Each core computes
q/k/v projections for its 128 columns of Wq/Wk/Wv, full attention for its two
heads, and a partial output projection against its 128 rows of Wo. The host
sums the 8 bf16 partials and adds (bv @ Wo + bo), which is exact because
sum_s attn[s]=1 lets the v-bias commute through attention and Wo.

Compute is bf16 on the PE with fp32 PSUM accumulation. Per-core layouts:
  qT/kT: [128(d_local), B*L]   — contraction-major for the scoresT matmuls
  v:     [s, 64]+ones column   — lhsT for attn@v; the ones column makes the
                                 PE emit the softmax denominator as row 64
  scoresT[s, l] per (b, l-chunk), both heads in one 2-bank PSUM tile so one
  ScalarE exp covers them (no max subtraction: scores ~ N(0,1)).

Normalization: reciprocal of the denominator ROW (DVE, [1,2,512]), broadcast
across partitions with a K=1 fp32r matmul (~0.2us PE), then one fused DVE
mul producing the bf16 o-proj lhsT. No gpsimd in the path.

Scheduling: scores run `lag` s-tiles ahead of attn@v. Chunk 0 interleaves
proj chunks 1-3 at s-tile 4/8/12 (attention starts right after proj 0);
batch-1 projections fill chunks 1-4; the previous chunk's o-projection is
spread over s-tiles 3-6 of the next chunk so output DMA streams evenly.
x tiles load on sync+vector DMA queues, weights on scalar+gpsimd, output
writes alternate sync/gpsimd.
"""

from contextlib import ExitStack

import ml_dtypes
import numpy as np

import concourse.bacc as bacc
import concourse.mybir as mybir
import concourse.tile as tile
from concourse.bass_utils import run_bass_kernel_spmd

D_MODEL = 1024
N_HEAD = 16
HEAD_DIM = 64
B = 2
L = 2048
N_CORES = 8
HPC = N_HEAD // N_CORES  # heads per core
MLOC = HPC * HEAD_DIM  # 128: local d width per core

F32 = mybir.dt.float32
F32R = mybir.dt.float32r
BF16 = mybir.dt.bfloat16
NPBF16 = ml_dtypes.bfloat16


def build_nc(Lb=L, lc_size=512, nch=512):
    """Build the per-core Bass program. Lb = sequence length per batch."""
    BLb = B * Lb
    KT = D_MODEL // 128  # 8 contraction tiles for the projections
    n_nch = BLb // nch  # projection column chunks
    st_per_nch = nch // 128  # s-tiles per projection chunk
    n_lc = Lb // lc_size  # attention l-chunks per batch
    n_st = Lb // 128  # s-tiles per batch
    n_lt = lc_size // 128  # l-tiles (128) per l-chunk

    nc = bacc.Bacc("TRN2", target_bir_lowering=False, debug=False)

    xT = nc.dram_tensor("xT", [D_MODEL, BLb], BF16, kind="ExternalInput").ap()
    wq = nc.dram_tensor("wq", [D_MODEL, MLOC], BF16, kind="ExternalInput").ap()
    wk = nc.dram_tensor("wk", [D_MODEL, MLOC], BF16, kind="ExternalInput").ap()
    wv = nc.dram_tensor("wv", [D_MODEL, MLOC], BF16, kind="ExternalInput").ap()
    wo = nc.dram_tensor("wo", [MLOC, D_MODEL], BF16, kind="ExternalInput").ap()
    bq = nc.dram_tensor("bq", [MLOC, 1], F32, kind="ExternalInput").ap()
    bk = nc.dram_tensor("bk", [MLOC, 1], F32, kind="ExternalInput").ap()
    out = nc.dram_tensor("out", [BLb, D_MODEL], BF16, kind="ExternalOutput").ap()

    with tile.TileContext(nc) as tc, ExitStack() as ctx:
        consts = ctx.enter_context(tc.tile_pool(name="consts", bufs=1))
        qk_sb = ctx.enter_context(tc.tile_pool(name="qk_sb", bufs=1))
        xt_pool = ctx.enter_context(tc.tile_pool(name="xt", bufs=3 * KT))
        # PSUM: big pool (2-bank slots x3) shared by scoresT / projections /
        # o-proj / rcp broadcast; av pool = two 1-bank tiles. Total 8 banks.
        big_ps = ctx.enter_context(tc.tile_pool(name="big_ps", bufs=3, space="PSUM"))
        av_ps = ctx.enter_context(tc.tile_pool(name="av_ps", bufs=1, space="PSUM"))
        exp_pool = ctx.enter_context(tc.tile_pool(name="expT", bufs=8))
        att_sb = ctx.enter_context(tc.tile_pool(name="att_sb", bufs=3))
        out_pool = ctx.enter_context(tc.tile_pool(name="out_sb", bufs=6))

        # x k-tiles load in PAIRS (256KB DMAs amortize the ~0.6us per-DMA
        # queue occupancy): pair j holds k-tiles 2j, 2j+1.
        xtr = xT.rearrange("(a p) n -> p a n", p=128)
        NP = KT // 2  # pairs per chunk

        def load_xts(nc_i, split=False):
            csl = slice(nc_i * nch, (nc_i + 1) * nch)
            pairs = []
            for j in range(NP):
                xt = xt_pool.tile([128, 2, nch], BF16, tag="xt", name="xt")
                eng = nc.gpsimd if split and j % 2 == 1 else nc.sync
                eng.dma_start(xt[:], xtr[:, 2 * j : 2 * j + 2, csl])
                pairs.append(xt)
            return [pairs[k // 2][:, k % 2, :] for k in range(KT)]

        # Weights resident in SBUF, one batched DMA per tensor: k-tile k of
        # w* at [:, k, :].
        wq_sb = consts.tile([128, KT, MLOC], BF16, tag="wq")
        wk_sb = consts.tile([128, KT, MLOC], BF16, tag="wk")
        wv_sb = consts.tile([128, KT, MLOC], BF16, tag="wv")
        wo_sb = consts.tile([128, D_MODEL], BF16, tag="wo")
        nc.gpsimd.dma_start(wq_sb[:], wq.rearrange("(k p) m -> p k m", p=128))
        nc.gpsimd.dma_start(wk_sb[:], wk.rearrange("(k p) m -> p k m", p=128))
        xts0 = load_xts(0, split=True)  # even pairs race the weights on sync
        bq_sb = consts.tile([MLOC, 1], F32, tag="bq")
        bk_sb = consts.tile([MLOC, 1], F32, tag="bk")
        nc.gpsimd.dma_start(bq_sb[:], bq)
        nc.gpsimd.dma_start(bk_sb[:], bk)
        nc.gpsimd.dma_start(wv_sb[:], wv.rearrange("(k p) m -> p k m", p=128))
        nc.gpsimd.dma_start(wo_sb[:], wo)

        # Persistent activations.
        qT_sb = qk_sb.tile([128, BLb], BF16, tag="qT")  # [d_local, b*Lb+l]
        kT_sb = qk_sb.tile([128, BLb], BF16, tag="kT")
        # v (natural layout) + ones column: per (b, head): [128, n_st, 65]
        vaug = [
            [qk_sb.tile([128, n_st, HEAD_DIM + 1], BF16, tag=f"vaug{bi}{h}",
                        name=f"vaug{bi}{h}")
             for h in range(HPC)]
            for bi in range(B)
        ]
        for bi in range(B):
            for h in range(HPC):
                nc.vector.memset(vaug[bi][h][:, :, HEAD_DIM:], 1.0)

        def proj_qk(nc_i, xts):
            """q/k projections for one column chunk of x."""
            csl = slice(nc_i * nch, (nc_i + 1) * nch)
            ps_qk = big_ps.tile([128, 2, nch], F32, tag="big", name="ps_qk")
            for k in range(KT):
                nc.tensor.matmul(ps_qk[:, 0, :], wq_sb[:, k, :], xts[k][:],
                                 start=(k == 0), stop=(k == KT - 1))
                nc.tensor.matmul(ps_qk[:, 1, :], wk_sb[:, k, :], xts[k][:],
                                 start=(k == 0), stop=(k == KT - 1))
            nc.vector.tensor_scalar_add(qT_sb[:, csl], ps_qk[:, 0, :], bq_sb[:])
            nc.vector.tensor_scalar_add(kT_sb[:, csl], ps_qk[:, 1, :], bk_sb[:])

        def proj_v(nc_i, xts):
            """v projection in natural [s, d_local] layout: lhsT = xT tiles."""
            ps_v = big_ps.tile([128, nch], F32, tag="big", name="ps_v")
            for st in range(st_per_nch):
                ssl = slice(128 * st, 128 * (st + 1))
                for k in range(KT):
                    nc.tensor.matmul(ps_v[:, ssl], xts[k][:, ssl],
                                     wv_sb[:, k, :],
                                     start=(k == 0), stop=(k == KT - 1))
            for st in range(st_per_nch):
                st_g = nc_i * st_per_nch + st
                bi, st_b = divmod(st_g, n_st)
                for h in range(HPC):
                    nc.vector.tensor_copy(
                        vaug[bi][h][:, st_b, :HEAD_DIM],
                        ps_v[:, 128 * st + HEAD_DIM * h
                             : 128 * st + HEAD_DIM * (h + 1)])

        def proj_chunk(nc_i, xts=None):
            if xts is None:
                xts = load_xts(nc_i)
            proj_qk(nc_i, xts)
            proj_v(nc_i, xts)

        def norm_chunk(ps_av, width):
            """Normalize one chunk's attn@v into the bf16 o-proj lhsT.
            den sits in row 64 of each head's av PSUM tile. gpsimd does the
            partition broadcast (it is otherwise idle); the PE is untouched."""
            avs = att_sb.tile([HEAD_DIM + 1, 2, lc_size], F32, tag="avs",
                              name="avs")
            den = att_sb.tile([1, 2, lc_size], F32, tag="den", name="den")
            for h in range(HPC):  # den first: it heads the bcast/recip chain
                nc.vector.tensor_copy(den[0:1, h, :width],
                                      ps_av[h][64:65, :width])
            for h in range(HPC):
                nc.vector.tensor_copy(avs[:, h, :width], ps_av[h][:, :width])
            bden = att_sb.tile([128, 2, lc_size], F32, tag="bden", name="bden")
            nc.gpsimd.partition_broadcast(bden[:, :, :width],
                                          den[0:1, :, :width])
            rcp = att_sb.tile([128, 2, lc_size], F32, tag="rcp", name="rcp")
            nc.vector.reciprocal_approx_fast(rcp[:, :, :width],
                                             bden[:, :, :width])
            oT = att_sb.tile([128, lc_size], BF16, tag="oT", name="oT", bufs=4)
            for h in range(HPC):
                hsl = slice(64 * h, 64 * (h + 1))
                nc.vector.tensor_mul(oT[hsl, :width],
                                     avs[:HEAD_DIM, h, :width],
                                     rcp[:HEAD_DIM, h, :width])
            return oT[:, :width]

        oproj_n = 0

        def oproj_tile(oT, bi, loff, lt, scalar_evac=False):
            """Output projection of one 128-token tile of a normalized chunk."""
            nonlocal oproj_n
            ps_o = big_ps.tile([128, 2, 512], F32, tag="big", name="ps_o")
            for dh in range(2):
                nc.tensor.matmul(ps_o[:, dh, :],
                                 oT[:, 128 * lt : 128 * (lt + 1)],
                                 wo_sb[:, 512 * dh : 512 * (dh + 1)],
                                 start=True, stop=True)
            ob = out_pool.tile([128, D_MODEL], BF16, tag="ob")
            if scalar_evac:  # tail only: ScalarE is idle once exps are done
                nc.scalar.copy(ob[:], ps_o.rearrange("p a b -> p (a b)"))
            else:
                nc.vector.tensor_copy(ob[:],
                                      ps_o.rearrange("p a b -> p (a b)"))
            oproj_n += 1
            nc.sync.dma_start(
                out[bi * Lb + loff + 128 * lt
                    : bi * Lb + loff + 128 * (lt + 1), :], ob[:])

        proj_qk(0, xts0)  # v-proj of chunk 0 is deferred past the first sc
        prefetched = {}

        def prefetch(p):
            if p < n_nch and p not in prefetched:
                prefetched[p] = load_xts(p)

        def proj_filler(p):
            proj_chunk(p, prefetched.pop(p, None))
            prefetch(p + 1)

        # Chunk list and insert schedule: proj chunk p must be emitted before
        # any sc(st) with st//st_per_nch == p%n_pb of the owning batch.
        chunks = [(bi, lc * lc_size, lc_size)
                  for bi in range(B) for lc in range(n_lc)]
        ppb = Lb // nch  # proj chunks per batch
        # Prefetches are deferred past the first matmuls: queued DMAs ahead of
        # a consumer inflate its queue-position semaphore wait.
        inserts = {(0, 1): [lambda: proj_v(0, xts0), lambda: prefetch(1)]}
        for p in range(1, ppb):  # batch-0 projs inside chunk 0, just in time
            inserts.setdefault((0, p * st_per_nch), []).append(
                lambda p=p: proj_filler(p))
        # batch-1 projs: one per chunk at st==3 for chunks 1..3, rest early in
        # chunk 4 before their kT s-range is consumed (st 3, before the oproj
        # window at st 8-11).
        b1 = list(range(ppb, n_nch))
        for ci in range(1, n_lc):
            if b1:
                p = b1.pop(0)
                inserts.setdefault((ci, 3), []).append(
                    lambda p=p: proj_filler(p))
        while b1:
            p = b1.pop(0)
            inserts.setdefault((n_lc, 3), []).append(
                lambda p=p: proj_filler(p))

        lag = 3  # av runs `lag` s-tiles behind scores
        oproj_q = []  # (oT, bi, loff) chunks awaiting o-projection
        for ci, (bi, loff, width) in enumerate(chunks):
            lsl = slice(bi * Lb + loff, bi * Lb + loff + width)
            ps_av = [av_ps.tile([HEAD_DIM + 1, lc_size], F32, tag=f"av{h}",
                                name=f"av{h}")[:, :width] for h in range(HPC)]
            exs = [None] * n_st

            def do_sc(st):
                ssl = slice(bi * Lb + st * 128, bi * Lb + (st + 1) * 128)
                ps_sc = big_ps.tile([128, HPC, lc_size], F32, tag="big",
                                    name="ps_sc")
                for h in range(HPC):
                    hsl = slice(64 * h, 64 * (h + 1))
                    nc.tensor.matmul(ps_sc[:, h, :width], kT_sb[hsl, ssl],
                                     qT_sb[hsl, lsl],
                                     start=True, stop=True,
                                     tile_position=(64 * h, 0))
                ex = exp_pool.tile([128, HPC, lc_size], BF16, tag="ex",
                                   name="ex")
                nc.scalar.activation(ex[:, :, :width], ps_sc[:, :, :width],
                                     mybir.ActivationFunctionType.Exp,
                                     scale=1.0 / np.sqrt(HEAD_DIM))
                exs[st] = ex

            def do_av(st):
                for h in range(HPC):
                    nc.tensor.matmul(ps_av[h][:], vaug[bi][h][:, st, :],
                                     exs[st][:, h, :width],
                                     start=(st == 0), stop=(st == n_st - 1))

            for st in range(n_st):
                for thunk in inserts.get((ci, st), []):
                    thunk()
                if oproj_q and 8 <= st < 8 + n_lt:
                    oproj_tile(*oproj_q[0][:3], st - 8)
                    if st - 8 == n_lt - 1:
                        oproj_q.pop(0)
                do_sc(st)
                if st >= lag:
                    do_av(st - lag)
            for st in range(n_st - lag, n_st):
                do_av(st)
            # Normalize this chunk now (cheap); o-projection happens early in
            # the next chunk so output DMA is spread across the kernel.
            oT = norm_chunk(ps_av, width)
            oproj_q.append((oT, bi, loff))
        while oproj_q:
            oT, bi, loff = oproj_q.pop(0)
            for lt in range(n_lt):
                oproj_tile(oT, bi, loff, lt, scalar_evac=(lt % 2 == 1))

    nc.compile()
    return nc


def make_in_maps(x, Wq, bq, Wk, bk, Wv, bv, Wo, Lb=L):
    """Per-core input dicts from full inputs."""
    BLb = B * Lb
    xT = np.ascontiguousarray(
        np.asarray(x, np.float32).reshape(BLb, D_MODEL).T).astype(NPBF16)
    Wq = np.asarray(Wq, np.float32).astype(NPBF16)
    Wk = np.asarray(Wk, np.float32).astype(NPBF16)
    Wv = np.asarray(Wv, np.float32).astype(NPBF16)
    Wo = np.asarray(Wo, np.float32).astype(NPBF16)
    in_maps = []
    for c in range(N_CORES):
        dsl = slice(MLOC * c, MLOC * (c + 1))
        in_maps.append({
            "xT": xT,
            "wq": np.ascontiguousarray(Wq[:, dsl]),
            "wk": np.ascontiguousarray(Wk[:, dsl]),
            "wv": np.ascontiguousarray(Wv[:, dsl]),
            "wo": np.ascontiguousarray(Wo[dsl, :]),
            "bq": np.ascontiguousarray(np.asarray(bq, np.float32)[dsl].reshape(MLOC, 1)),
            "bk": np.ascontiguousarray(np.asarray(bk, np.float32)[dsl].reshape(MLOC, 1)),
        })
    return in_maps


_NC_CACHE = {}


def _get_nc():
    if "nc" not in _NC_CACHE:
        _NC_CACHE["nc"] = build_nc()
    return _NC_CACHE["nc"]


def kernel(x, Wq, bq, Wk, bk, Wv, bv, Wo, bo):
    nc = _get_nc()
    in_maps = make_in_maps(x, Wq, bq, Wk, bk, Wv, bv, Wo)
    res = run_bass_kernel_spmd(nc, in_maps, list(range(N_CORES)))
    acc = np.zeros((B * L, D_MODEL), dtype=np.float32)
    for c in range(N_CORES):
        acc += np.asarray(res.results[c]["out"], dtype=np.float32)
    # v-bias and o-bias commute through attention (sum_s attn=1) and Wo.
    acc += (np.asarray(bv, np.float32) @ np.asarray(Wo, np.float32)
            + np.asarray(bo, np.float32))
    return acc.reshape(B, L, D_MODEL)
